# revision 1
# baseline (speedup 1.0000x reference)
"""CosSim2d Trainium2 kernel (8 NeuronCores, batch-sharded).

Computation (per image): 3x3 patches of x (pad 1) are L2-normalized per
channel over the 9 taps, contracted with L2-normalized weights over
(cin, tap), then sign-preserving pow with p (p==1 in practice).

Mapping per core (4 images, processed as 2 image-pairs on 128 partitions):
  - x loaded host-padded (66x66) and DMA-cast fp32->fp16 (imgA on
    partitions 64:128, imgB on 0:64).
  - sq = x^2 on ScalarE (fp16).
  - patch-norm^2 = 3x3 box filter of sq: computed on the TensorEngine as 9
    identity-weight matmuls accumulating in PSUM ("same-half" quads
    (0,0)/(64,64)).
  - y = rsqrt(norm^2) via ACT Ln(+eps bias) then Exp(scale=-0.5) -> fp16.
  - z_l = shift_l(x) * y: 9 fp16 tensor_tensor muls, 8 on the DVE (2x mode;
    a +1-shifted copy B of x keeps all access patterns 4B-aligned) and one
    on GpSimd (contention-free with DVE tensor_tensor).
  - s = sum_l Wn_l^T @ z_l: matmuls accumulating in PSUM ("cross-half"
    quads (64,0)/(0,64) - all four 64x64 PE quadrants stay busy).
  - drain PSUM -> SBUF on ScalarE, DMA out fp32.
"""

import numpy as np

import concourse.bass as bass
import concourse.tile as tile
from concourse import mybir
from concourse.bass_utils import run_bass_kernel_spmd

F32 = mybir.dt.float32
F16 = mybir.dt.float16

N_CORES = 8
N, CIN, HW = 32, 64, 64
COUT, KS = 64, 3
NLOC = N // N_CORES            # images per core
NPAIR = NLOC // 2              # image pairs per core
PH = HW + 2                    # padded row length (66)
PADDED = PH * PH               # 4356
IMG = HW * HW                  # 4096
CHUNK = 1024                   # output cols per pipeline chunk (16 rows)
ROWS_PER_CHUNK = CHUNK // HW   # 16
NCHUNK = IMG // CHUNK          # 4
EPS = 1e-12
RSQ_BIAS = 1e-20               # y = rsqrt(ssum + bias): bias ~ eps^2 clamp


def _split_excess_waits(nc, max_waits=1):
    """This container's walrus accepts only one sync-wait per instruction;
    move extra waits onto preceding same-engine NOPs."""
    fn = nc.m.functions[0]

    def fix_block(bb):
        if hasattr(bb, "blocks"):
            for sub in bb.blocks:
                fix_block(sub)
        if not hasattr(bb, "instructions"):
            return
        new_list = []
        changed = False
        for ins in bb.instructions:
            si = ins.sync_info
            if si is not None and si.on_wait is not None and len(si.on_wait) > max_waits:
                waits = list(si.on_wait)
                k = 0
                while len(waits) > max_waits:
                    chunk, waits = waits[:max_waits], waits[max_waits:]
                    nop = mybir.InstNoOp(
                        name=f"{ins.name}_wsplit{k}", engine=ins.engine, ins=[], outs=[]
                    )
                    nop.sync_info = mybir.SyncInfo(on_wait=chunk, on_update=[])
                    new_list.append(nop)
                    k += 1
                ins.sync_info = mybir.SyncInfo(
                    on_wait=waits, on_update=list(si.on_update or [])
                )
                changed = True
            new_list.append(ins)
        if changed:
            bb.instructions = new_list

    for bb in fn.blocks:
        fix_block(bb)


TAPS = [(di, dj) for di in range(3) for dj in range(3)]  # shift-1 offsets


def _body(nc, tc, ctx, xp_in, wt_in, ident_in, out_t):
    # ---- weight prep (tiny, fp32) -------------------------------------
    wpool = ctx.enter_context(tc.tile_pool(name="w", bufs=1))
    wt32 = wpool.tile([128, 576], F32)
    nc.sync.dma_start(wt32[0:64, :], wt_in[:, :])
    nc.sync.dma_start(wt32[64:128, :], wt_in[:, :])
    ident = wpool.tile([128, 64], F16)
    nc.sync.dma_start(ident[:, :], ident_in[:, :])

    wsq = wpool.tile([128, 576], F32)
    nc.scalar.activation(wsq[:, :], wt32[:, :], mybir.ActivationFunctionType.Square)
    nrm2 = wpool.tile([128, 64], F32)
    wsq_vl = wsq[:, :].rearrange("p (l v) -> p v l", l=9)
    nc.vector.tensor_reduce(
        nrm2[:, :], wsq_vl, axis=mybir.AxisListType.X, op=mybir.AluOpType.add
    )
    rsq_bias = wpool.tile([128, 1], F32)
    nc.vector.memset(rsq_bias[:, :], RSQ_BIAS)
    nrm = wpool.tile([128, 64], F32)
    nc.scalar.activation(nrm[:, :], nrm2[:, :], mybir.ActivationFunctionType.Sqrt)
    nrmc = wpool.tile([128, 64], F32)
    nc.vector.tensor_scalar_max(nrmc[:, :], nrm[:, :], EPS)
    rinv = wpool.tile([128, 64], F32)
    nc.vector.reciprocal(rinv[:, :], nrmc[:, :])
    wn16 = wpool.tile([128, 576], F16)
    nc.vector.tensor_tensor(
        wn16[:, :].rearrange("p (l v) -> p l v", l=9),
        wt32[:, :].rearrange("p (l v) -> p l v", l=9),
        rinv[:, :].unsqueeze(1).to_broadcast((128, 9, 64)),
        op=mybir.AluOpType.mult,
    )
    wn_r = wn16[:, :].rearrange("p (l v) -> p l v", l=9)

    # ---- pools for the main pipeline ----------------------------------
    xpool = ctx.enter_context(tc.tile_pool(name="x", bufs=2))
    bpool = ctx.enter_context(tc.tile_pool(name="b", bufs=2))
    sqpool = ctx.enter_context(tc.tile_pool(name="sq", bufs=2))
    ypool = ctx.enter_context(tc.tile_pool(name="y", bufs=3))
    lnpool = ctx.enter_context(tc.tile_pool(name="ln", bufs=3))
    zpool = ctx.enter_context(tc.tile_pool(name="z", bufs=3))
    opool = ctx.enter_context(tc.tile_pool(name="o", bufs=2))
    ps_box = ctx.enter_context(tc.tile_pool(name="psbox", bufs=2, space="PSUM"))
    ps_s = ctx.enter_context(tc.tile_pool(name="pss", bufs=2, space="PSUM"))

    for tp in range(NPAIR):
        x16 = xpool.tile([128, PADDED], F16)
        # One full-width 128-partition DMA (half-partition pairs would
        # serialize on the DMA rings). The host pre-swaps each image pair so
        # imgA (= image 2*tp) lands on partitions 64:128 and imgB on 0:64 -
        # the swap makes the four matmul quadrant assignments below disjoint.
        nc.gpsimd.dma_start(                                      # cast f32->f16
            x16[:, :], xp_in[2 * tp : 2 * tp + 2].rearrange("n c m -> (n c) m")
        )
        b16 = bpool.tile([128, PADDED], F16)
        nc.sync.dma_start(b16[:, 0 : PADDED - 1], x16[:, 1:PADDED])  # B[k]=A[k+1]
        sq16 = sqpool.tile([128, PADDED], F16)
        nc.scalar.activation(
            sq16[:, :], x16[:, :], mybir.ActivationFunctionType.Square
        )

        x_r = x16[:, :].rearrange("p (r c) -> p r c", r=PH)
        b_r = b16[:, :].rearrange("p (r c) -> p r c", r=PH)
        sq_r = sq16[:, :].rearrange("p (r c) -> p r c", r=PH)

        out32 = opool.tile([128, IMG], F32)

        for k in range(NCHUNK):
            R = k * ROWS_PER_CHUNK
            # ---- box filter on the PE: ssum = sum_l shift_l(sq) -------
            # (lo/hi emitted alternating so the two quads' LDWEIGHTS and
            # matmuls overlap in the PE queue)
            ssum = ps_box.tile([128, CHUNK], F32)
            for sub in range(CHUNK // 512):
                r0 = R + 8 * sub
                for l, (di, dj) in enumerate(TAPS):
                    for (lo, hi), tpos in (((0, 64), (0, 0)), ((64, 128), (64, 64))):
                        nc.tensor.matmul(
                            ssum[lo:hi, sub * 512 : (sub + 1) * 512],
                            ident[lo:hi, :],
                            sq_r[lo:hi, r0 + di : r0 + di + 8, dj : dj + 64],
                            start=(l == 0),
                            stop=(l == 8),
                            tile_position=tpos,
                        )
            # ---- y = rsqrt(ssum + eps) = exp(-0.5*ln(ssum + eps)) -----
            ln32 = lnpool.tile([128, CHUNK], F32)
            nc.scalar.activation(
                ln32[:, :],
                ssum[:, :],
                mybir.ActivationFunctionType.Ln,
                bias=rsq_bias[:, :],
            )
            y16 = ypool.tile([128, CHUNK], F16)
            nc.scalar.activation(
                y16[:, :], ln32[:, :], mybir.ActivationFunctionType.Exp, scale=-0.5
            )
            y_v = y16[:, :].rearrange("p (r c) -> p r c", r=ROWS_PER_CHUNK)

            # ---- z_l = shift_l(x) * y on the DVE (fp16 2x mode) -------
            z = zpool.tile([128, 9 * CHUNK], F16)
            z_r = z[:, :].rearrange("p (l n) -> p l n", l=9)
            for l, (di, dj) in enumerate(TAPS):
                if dj == 1:
                    src = b_r[:, R + di : R + di + ROWS_PER_CHUNK, 0:64]
                else:
                    src = x_r[:, R + di : R + di + ROWS_PER_CHUNK, dj : dj + 64]
                # two taps ride on GpSimd (tensor_tensor never contends with
                # the DVE's SBUF ports), the rest on the DVE in 2x fp16 mode
                eng = nc.gpsimd if l in (1, 4) else nc.vector
                eng.tensor_tensor(z_r[:, l, :], src, y_v, op=mybir.AluOpType.mult)

            # ---- conv: s = sum_l Wn_l^T @ z_l on the PE ---------------
            s_ps = ps_s.tile([128, CHUNK], F32)
            for sub in range(CHUNK // 512):
                for l in range(9):
                    for (zlo, zhi), (olo, ohi), tpos in (
                        ((64, 128), (0, 64), (64, 0)),
                        ((0, 64), (64, 128), (0, 64)),
                    ):
                        nc.tensor.matmul(
                            s_ps[olo:ohi, sub * 512 : (sub + 1) * 512],
                            wn_r[zlo:zhi, l, :],
                            z_r[zlo:zhi, l, sub * 512 : (sub + 1) * 512],
                            start=(l == 0),
                            stop=(l == 8),
                            tile_position=tpos,
                        )
            # ---- drain PSUM -> SBUF (note: s of imgA lands on 0:64) ---
            nc.scalar.copy(out32[:, k * CHUNK : (k + 1) * CHUNK], s_ps[:, :])

        # out32 partitions 0:64 hold image 2*tp (the conv cross-quads undo
        # the input swap), so one full-width DMA writes both images in order
        nc.sync.dma_start(
            out_t[2 * tp : 2 * tp + 2].rearrange("n c h w -> (n c) (h w)"),
            out32[:, :],
        )


def _build():
    nc = bass.Bass(
        "TRN2", target_bir_lowering=False, debug=False, num_devices=N_CORES
    )
    xp_in = nc.dram_tensor("xp", [NLOC, CIN, PADDED], F32, kind="ExternalInput").ap()
    wt_in = nc.dram_tensor("wt", [CIN, 9 * COUT], F32, kind="ExternalInput").ap()
    ident_in = nc.dram_tensor("ident", [128, 64], F16, kind="ExternalInput").ap()
    out_t = nc.dram_tensor(
        "out", [NLOC, COUT, HW, HW], F32, kind="ExternalOutput"
    ).ap()
    from contextlib import ExitStack

    with tile.TileContext(nc) as tc, ExitStack() as ctx:
        _body(nc, tc, ctx, xp_in, wt_in, ident_in, out_t)
    _split_excess_waits(nc, 1)
    return nc


_CACHE = {}


def _get_program():
    if "nc" not in _CACHE:
        _CACHE["nc"] = _build()
    return _CACHE["nc"]


def kernel(x, w, p):
    x = np.asarray(x, dtype=np.float32)
    w = np.asarray(w, dtype=np.float32)
    p = np.asarray(p, dtype=np.float32)
    assert x.shape == (N, CIN, HW, HW) and w.shape == (COUT, CIN, 9)
    nc = _get_program()

    xp = np.zeros((N, CIN, PH, PH), dtype=np.float32)
    xp[:, :, 1:-1, 1:-1] = x
    xp = xp.reshape(N, CIN, PADDED)
    # (v, c, l) -> (c, l, v), flattened to (c, 9*64) for direct lhsT slices
    wt = np.ascontiguousarray(np.transpose(w.astype(np.float32), (1, 2, 0))).reshape(
        CIN, 9 * COUT
    )
    ident = np.concatenate([np.eye(64), np.eye(64)], axis=0).astype(np.float16)

    # swap each image pair so the single full-width in-DMA lands imgA on
    # SBUF partitions 64:128 (pure host-side marshaling)
    pair_swap = [1, 0, 3, 2]
    in_maps = [
        {
            "xp": np.ascontiguousarray(xp[c * NLOC : (c + 1) * NLOC][pair_swap]),
            "wt": wt,
            "ident": ident,
        }
        for c in range(N_CORES)
    ]
    res = run_bass_kernel_spmd(nc, in_maps, list(range(N_CORES)))
    out = np.concatenate([res.results[c]["out"] for c in range(N_CORES)], axis=0)

    if not np.allclose(p, 1.0):
        # generic sign-preserving pow fallback (p is ones for this problem's
        # setup_inputs, so this branch is never taken in practice)
        out = np.sign(out) * (np.abs(out) + EPS) ** p[None, :, None, None]
    return out.astype(np.float32)

